# revision 1
# baseline (speedup 1.0000x reference)
"""Trainium2 Bass kernel for a pre-LN transformer block (B=128,T=256,C=384,H=6,D=64).

Data-parallel over batch across 8 NeuronCores (16 batches/core), processed in
pairs so the QKV and FFN1 matmuls stream a 512-wide moving operand (two
batches' tokens side by side). All matmuls run as float32r. LN gamma/beta are
folded into the weights on the host; device LN is (x - mu) * rstd via
bn_stats/bn_aggr. Attention uses the transposed-score orientation ([S,T]):
softmax denominators come from an all-ones matmul that also broadcasts them
across partitions, causal masking zeroes exp(scores) with
gpsimd.affine_select, and normalization happens during the attn@v PSUM
evacuation.
"""

import sys

if "/opt/trn_rl_repo" not in sys.path:
    sys.path.insert(0, "/opt/trn_rl_repo")

import numpy as np

import concourse.bass as bass
import concourse.mybir as mybir
import concourse.tile as tile
from concourse import bacc

# All ACT functions used here (Exp, Ln, Relu, Identity, Copy) live in the
# 'natural_log_exp_and_others' table set. Blank the other sets (preserving
# dict order, which defines act_func_set_id) so the table-load fixpoint
# settles on a single ACT_TABLE_LOAD instead of thrashing sets per batch.
_KEEP_ACT_SET = "natural_log_exp_and_others"
_orig_get_act_tables = bacc.get_activation_tables


def _one_set_tables(arch):
    t = _orig_get_act_tables(arch)
    assert _KEEP_ACT_SET in t
    return {k: (v if k == _KEEP_ACT_SET else set()) for k, v in t.items()}


bacc.get_activation_tables = _one_set_tables

F32 = mybir.dt.float32
F32R = mybir.dt.float32r
AF = mybir.ActivationFunctionType
ALU = mybir.AluOpType

B, T, C, H, D = 128, 256, 384, 6, 64
NCORES = 8
BL = B // NCORES          # batches per core
F = 4 * C                 # 1536
P = 128
TCH = T // P              # 2 token chunks
CCH = C // P              # 3 channel chunks
FCH = F // P              # 12 ffn chunks
HD = H * D                # 384
SCALE = float(C) ** -0.5  # reference scales by full model dim
EPS = 1e-5


def build_program(bl=BL, flags=frozenset(), repeat=1,
                  tr_split=False, tr_bufs=1, ps1_bufs=6, wk_bufs=2):
    """Per-core Bass program. `flags` lists nonzero bias terms
    ('qb','kb','vb','bo','b1','b2'). `repeat` wraps the whole computation in
    a hardware loop (benchmarking only)."""
    assert bl % 2 == 0
    use_qb = "qb" in flags
    use_kb = "kb" in flags
    use_vb = "vb" in flags
    use_bo = "bo" in flags
    use_b1 = "b1" in flags
    use_b2 = "b2" in flags

    nc = bacc.Bacc("TRN2", target_bir_lowering=False, debug=False,
                   num_devices=NCORES)

    x_d = nc.dram_tensor("x", [bl, T, C], F32, kind="ExternalInput")
    wq_d = nc.dram_tensor("wq", [P, CCH, HD], F32R, kind="ExternalInput")
    wk_d = nc.dram_tensor("wk", [P, CCH, HD], F32R, kind="ExternalInput")
    wv_d = nc.dram_tensor("wv", [P, CCH, HD], F32R, kind="ExternalInput")
    qb_d = nc.dram_tensor("qb", [P, CCH], F32, kind="ExternalInput")
    kb_d = nc.dram_tensor("kb", [P, CCH], F32, kind="ExternalInput")
    vb_d = nc.dram_tensor("vb", [1, HD], F32R, kind="ExternalInput")
    wo_d = nc.dram_tensor("wo", [D, H, C], F32R, kind="ExternalInput")
    bo_d = nc.dram_tensor("bo", [1, C], F32R, kind="ExternalInput")
    w1_d = nc.dram_tensor("w1", [P, CCH, F], F32R, kind="ExternalInput")
    b1_d = nc.dram_tensor("b1c", [P, FCH], F32, kind="ExternalInput")
    w2_d = nc.dram_tensor("w2", [P, FCH, C], F32R, kind="ExternalInput")
    b2_d = nc.dram_tensor("b2", [1, C], F32R, kind="ExternalInput")
    id_d = nc.dram_tensor("ident", [P, P], F32R, kind="ExternalInput")
    on_d = nc.dram_tensor("onesm", [P, P], F32R, kind="ExternalInput")
    tl_d = nc.dram_tensor("trilm", [P, P], F32R, kind="ExternalInput")
    ng_d = nc.dram_tensor("negm", [P, TCH, T], F32R, kind="ExternalInput")
    y_d = nc.dram_tensor("y", [bl, T, C], F32, kind="ExternalOutput")

    with tile.TileContext(nc) as tc:
        import contextlib
        with (
            tc.tile_pool(name="wpool", bufs=1) as wp,
            tc.tile_pool(name="work", bufs=wk_bufs) as wk_pool,
            tc.tile_pool(name="big", bufs=1) as bigp,
            (contextlib.nullcontext(None) if tr_split else
             tc.tile_pool(name="ps_tr", bufs=tr_bufs, space="PSUM")) as pstr,
            tc.tile_pool(name="ps_one", bufs=ps1_bufs, space="PSUM") as ps1,
        ):
            # ---- load weights/constants once ----
            wq = wp.tile([P, CCH, HD], F32R)
            wkk = wp.tile([P, CCH, HD], F32R)
            wv = wp.tile([P, CCH, HD], F32R)
            wo = wp.tile([D, H, C], F32R)
            w1 = wp.tile([P, CCH, F], F32R)
            w2 = wp.tile([P, FCH, C], F32R)
            ident = wp.tile([P, P], F32R)
            ones_t = wp.tile([P, P], F32R)
            trilm = wp.tile([P, P], F32R)
            negm = wp.tile([P, TCH, T], F32R)
            epsb = wp.tile([P, 1], F32)
            nc.gpsimd.memset(epsb[:], EPS)
            nc.sync.dma_start(wq[:], wq_d[:])
            nc.sync.dma_start(wkk[:], wk_d[:])
            nc.sync.dma_start(wv[:], wv_d[:])
            nc.sync.dma_start(wo[:], wo_d[:])
            nc.sync.dma_start(w1[:], w1_d[:])
            nc.sync.dma_start(w2[:], w2_d[:])
            nc.sync.dma_start(ident[:], id_d[:])
            nc.sync.dma_start(ones_t[:], on_d[:])
            nc.sync.dma_start(trilm[:], tl_d[:])
            nc.sync.dma_start(negm[:], ng_d[:])
            qb = kb = vb = bo = b1c = b2 = None
            if use_qb:
                qb = wp.tile([P, CCH], F32)
                nc.sync.dma_start(qb[:], qb_d[:])
            if use_kb:
                kb = wp.tile([P, CCH], F32)
                nc.sync.dma_start(kb[:], kb_d[:])
            if use_vb:
                vb = wp.tile([1, HD], F32R)
                nc.sync.dma_start(vb[:], vb_d[:])
            if use_bo:
                bo = wp.tile([1, C], F32R)
                nc.sync.dma_start(bo[:], bo_d[:])
            if use_b1:
                b1c = wp.tile([P, FCH], F32)
                nc.sync.dma_start(b1c[:], b1_d[:])
            if use_b2:
                b2 = wp.tile([1, C], F32R)
                nc.sync.dma_start(b2[:], b2_d[:])

            def layer_norm_T(src, dstT, i, evac_act):
                """src: [P, TCH, C] tokens-major tile. Writes (src-mu)*rstd
                transposed into dstT[:, :, i, :] ([P, CCH, 2, T] pair tile)."""
                st6 = wk_pool.tile([P, TCH, 6], F32, tag=f"st6_{i}")
                mv = wk_pool.tile([P, TCH, 2], F32, tag=f"mv_{i}")
                rstd = wk_pool.tile([P, TCH], F32, tag=f"rstd_{i}")
                for tch in range(TCH):
                    nc.vector.bn_stats(st6[:, tch, :], src[:, tch, :])
                    nc.vector.bn_aggr(mv[:, tch, :], st6[:, tch, :])
                # rstd = exp(-0.5 * ln(var + eps))
                nc.scalar.activation(rstd[:], mv[:, :, 1], AF.Ln, bias=epsb[:])
                nc.scalar.activation(rstd[:], rstd[:], AF.Exp, scale=-0.5)
                xn = wk_pool.tile([P, TCH, C], F32R, tag=f"xn_{i}", bufs=1)
                for tch in range(TCH):
                    nc.vector.tensor_scalar(
                        xn[:, tch, :], src[:, tch, :],
                        mv[:, tch, 0:1], rstd[:, tch:tch + 1],
                        ALU.subtract, ALU.mult,
                    )
                if tr_split:
                    trA = ps1.tile([P, 2, T], F32R, tag="ps1", name="trA")
                    trB = ps1.tile([P, T], F32R, tag="ps1", name="trB")

                    def _trdst(cc):
                        return trB if cc == 2 else trA[:, cc, :]
                else:
                    tr = pstr.tile([P, CCH, T], F32R, tag="tr")

                    def _trdst(cc):
                        return tr[:, cc, :]
                for tch in range(TCH):
                    for cc in range(CCH):
                        nc.tensor.transpose(
                            _trdst(cc)[:, tch * P:(tch + 1) * P],
                            xn[:, tch, cc * P:(cc + 1) * P],
                            ident[:],
                        )
                if tr_split:
                    if evac_act:
                        nc.scalar.copy(dstT[:, 0:2, i, :], trA[:])
                        nc.scalar.copy(dstT[:, 2, i, :], trB[:])
                    else:
                        nc.vector.tensor_copy(dstT[:, 0:2, i, :], trA[:])
                        nc.vector.tensor_copy(dstT[:, 2, i, :], trB[:])
                elif evac_act:
                    nc.scalar.copy(dstT[:, :, i, :], tr[:])
                else:
                    nc.vector.tensor_copy(dstT[:, :, i, :], tr[:])

            def body():
                for pb in range(bl // 2):
                    bp = (2 * pb, 2 * pb + 1)
                    xts = []
                    xnT2 = wk_pool.tile([P, CCH, 2, T], F32R, tag="xnT2")
                    for i, b in enumerate(bp):
                        xt = wk_pool.tile([P, TCH, C], F32, tag=f"xt{i}")
                        nc.sync.dma_start(
                            xt[:], x_d[b].rearrange("(tc p) c -> p tc c", p=P))
                        xts.append(xt)
                        layer_norm_T(xt, xnT2, i, evac_act=(i == 0))

                    # ---- q,k transposed [hd, (b,t)]; v natural [s, hd] ----
                    qsb2 = wk_pool.tile([P, CCH, 2, T], F32R, tag="qsb2")
                    ksb2 = wk_pool.tile([P, CCH, 2, T], F32R, tag="ksb2", bufs=1)
                    for wmat, bias_t, use_b, dst, eng in (
                        (wq, qb, use_qb, qsb2, "act"),
                        (wkk, kb, use_kb, ksb2, "dve"),
                    ):
                        for mc in range(CCH):
                            pp = ps1.tile([P, 2, T], F32, tag="ps1")
                            for kc in range(CCH):
                                nc.tensor.matmul(
                                    pp[:, :, :],
                                    wmat[:, kc, mc * P:(mc + 1) * P],
                                    xnT2[:, kc, :, :],
                                    start=(kc == 0), stop=(kc == CCH - 1),
                                )
                            if use_b:
                                nc.scalar.activation(
                                    dst[:, mc, :, :], pp[:], AF.Identity,
                                    bias=bias_t[:, mc:mc + 1])
                            elif eng == "act":
                                nc.scalar.copy(dst[:, mc, :, :], pp[:])
                            else:
                                nc.vector.tensor_copy(dst[:, mc, :, :], pp[:])

                    vsbs = []
                    for i in range(2):
                        vsb = wk_pool.tile([P, TCH, HD], F32R, tag=f"vsb{i}")
                        vsbs.append(vsb)
                        for sc in range(TCH):
                            vp = ps1.tile([P, HD], F32, tag="ps1")
                            for kc in range(CCH):
                                nc.tensor.matmul(
                                    vp[:, :],
                                    xnT2[:, kc, i, sc * P:(sc + 1) * P],
                                    wv[:, kc, :],
                                    start=(kc == 0),
                                    stop=(kc == CCH - 1 and not use_vb),
                                )
                            if use_vb:
                                nc.tensor.matmul(
                                    vp[:, :], ones_t[0:1, :], vb[0:1, :],
                                    start=False, stop=True)
                            if sc == 0:
                                nc.scalar.copy(vsb[:, sc, :], vp[:])
                            else:
                                nc.vector.tensor_copy(vsb[:, sc, :], vp[:])

                    # ---- attention per batch ----
                    xnews = []
                    for i, b in enumerate(bp):
                        e_all = bigp.tile([P, TCH, H, T], F32R, tag="e_all")
                        rbc = bigp.tile([P, H, T], F32, tag="rbc")
                        osb = wk_pool.tile([64, H, T], F32R, tag="osb", bufs=1)
                        for h in range(H):
                            hc, ho = h // 2, 64 * (h % 2)
                            sp = ps1.tile([P, TCH, T], F32, tag="ps1")
                            for sc in range(TCH):
                                nc.tensor.matmul(
                                    sp[:, sc, :],
                                    ksb2[ho:ho + D, hc, i, sc * P:(sc + 1) * P],
                                    qsb2[ho:ho + D, hc, i, :],
                                    start=True, stop=False,
                                )
                                # causal mask fused on PE: adds
                                # -1e30*max(0, s-t), so exp flushes to 0
                                nc.tensor.matmul(
                                    sp[:, sc, :], trilm[:, :], negm[:, sc, :],
                                    start=False, stop=True,
                                )
                            nc.scalar.activation(
                                e_all[:, :, h, :], sp[:], AF.Exp, scale=SCALE)
                        for pc in range(H // 2):
                            dp = ps1.tile([P, 2, T], F32, tag="ps1")
                            ops = []
                            for j in range(2):
                                h = 2 * pc + j
                                for sc in range(TCH):
                                    nc.tensor.matmul(
                                        dp[:, j, :], ones_t[:],
                                        e_all[:, sc, h, :],
                                        start=(sc == 0), stop=(sc == TCH - 1))
                                op_h = ps1.tile([D, T], F32, tag="ps1")
                                ops.append(op_h)
                                for sc in range(TCH):
                                    nc.tensor.matmul(
                                        op_h[:, :],
                                        vsbs[i][:, sc, h * D:(h + 1) * D],
                                        e_all[:, sc, h, :],
                                        start=(sc == 0), stop=(sc == TCH - 1))
                            nc.vector.reciprocal(
                                rbc[:, 2 * pc:2 * pc + 2, :], dp[:])
                            for j in range(2):
                                h = 2 * pc + j
                                nc.vector.tensor_tensor(
                                    osb[:, h, :], ops[j][:, :], rbc[0:D, h, :],
                                    ALU.mult,
                                )

                        # ---- out proj + residual ----
                        xnew = wk_pool.tile([P, TCH, C], F32, tag=f"xnew{i}")
                        xnews.append(xnew)
                        for tcc in range(TCH):
                            ap_t = ps1.tile([P, C], F32, tag="ps1")
                            for h in range(H):
                                nc.tensor.matmul(
                                    ap_t[:, :],
                                    osb[:, h, tcc * P:(tcc + 1) * P],
                                    wo[:, h, :],
                                    start=(h == 0),
                                    stop=(h == H - 1 and not use_bo))
                            if use_bo:
                                nc.tensor.matmul(
                                    ap_t[:, :], ones_t[0:1, :], bo[0:1, :],
                                    start=False, stop=True)
                            nc.vector.tensor_tensor(
                                xnew[:, tcc, :], ap_t[:, :], xts[i][:, tcc, :],
                                ALU.add)

                    # ---- LN2 -> xn2T pair ----
                    xn2T2 = wk_pool.tile([P, CCH, 2, T], F32R, tag="xn2T2")
                    for i in range(2):
                        layer_norm_T(xnews[i], xn2T2, i, evac_act=(i == 1))

                    # ---- FFN fused over the pair, streamed per f-chunk:
                    # hT chunk -> relu -> immediately accumulated into the
                    # four (batch, t-chunk) FFN2 output psums ----
                    fps = []
                    for j in range(4):
                        fp_j = ps1.tile([P, C], F32, tag="ps1", name=f"fp{j}")
                        fps.append(fp_j)
                    for mo in range(FCH):
                        hp = ps1.tile([P, 2, T], F32, tag="ps1")
                        for kc in range(CCH):
                            nc.tensor.matmul(
                                hp[:, :, :],
                                w1[:, kc, mo * P:(mo + 1) * P],
                                xn2T2[:, kc, :, :],
                                start=(kc == 0), stop=(kc == CCH - 1))
                        hsm = wk_pool.tile([P, 2, T], F32R, tag="hsm")
                        if use_b1:
                            nc.scalar.activation(
                                hsm[:], hp[:], AF.Relu, bias=b1c[:, mo:mo + 1])
                        else:
                            nc.scalar.activation(hsm[:], hp[:], AF.Relu)
                        for i in range(2):
                            for tcc in range(TCH):
                                nc.tensor.matmul(
                                    fps[2 * i + tcc][:, :],
                                    hsm[:, i, tcc * P:(tcc + 1) * P],
                                    w2[:, mo, :],
                                    start=(mo == 0),
                                    stop=(mo == FCH - 1 and not use_b2))

                    for i, b in enumerate(bp):
                        yout = wk_pool.tile([P, TCH, C], F32, tag=f"yout{i}")
                        for tcc in range(TCH):
                            fp = fps[2 * i + tcc]
                            if use_b2:
                                nc.tensor.matmul(
                                    fp[:, :], ones_t[0:1, :], b2[0:1, :],
                                    start=False, stop=True)
                            nc.vector.tensor_tensor(
                                yout[:, tcc, :], fp[:, :], xnews[i][:, tcc, :],
                                ALU.add)
                        nc.sync.dma_start(
                            y_d[b].rearrange("(tc p) c -> p tc c", p=P),
                            yout[:])

            if repeat > 1:
                with tc.For_i(0, repeat, 1):
                    body()
            else:
                body()

    nc.compile()
    return nc


def _make_negm():
    # negm[j, sc, t] moving operand; with trilm (lhsT[j, s] = 1 iff j <= s)
    # the accumulated matmul adds -BIG * #{j: j <= s_blk and cond(j, t)},
    # nonzero exactly where global s > t.
    BIG = np.float32(1e30)
    f32 = np.float32
    m = np.zeros((P, TCH, T), dtype=f32)
    jgt = np.tril(np.ones((P, P), dtype=f32), -1)  # [j, t] = 1 iff j > t
    m[:, 0, 0:P] = -BIG * jgt          # diagonal block of s-chunk 0
    m[:, 1, 0:P] = -BIG                # s-chunk 1 vs t-chunk 0: all masked
    m[:, 1, P:2 * P] = -BIG * jgt      # diagonal block of s-chunk 1
    return m


def prep_weights(Wq, Wk, Wv, Wo, bo, W1, b1, W2, b2, g1, be1, g2, be2):
    """Fold LN gamma/beta into projection weights; rearrange to SBUF layouts."""
    f32 = np.float32

    def kchunk(w, kdim):  # [K, M] -> [P, K//P, M]
        m = w.shape[1]
        return np.ascontiguousarray(
            w.reshape(kdim // P, P, m).transpose(1, 0, 2)).astype(f32)

    Wq2 = Wq.transpose(1, 0, 2).reshape(C, HD)
    Wk2 = Wk.transpose(1, 0, 2).reshape(C, HD)
    Wv2 = Wv.transpose(1, 0, 2).reshape(C, HD)
    out = {
        "wq": kchunk(g1[:, None] * Wq2, C),
        "wk": kchunk(g1[:, None] * Wk2, C),
        "wv": kchunk(g1[:, None] * Wv2, C),
        "wo": np.ascontiguousarray(
            Wo.reshape(H, D, C).transpose(1, 0, 2)).astype(f32),
        "w1": kchunk(g2[:, None] * W1, C),
        "w2": kchunk(W2, F),
        "ident": np.eye(P, dtype=f32),
        "onesm": np.ones((P, P), dtype=f32),
        "trilm": np.tril(np.ones((P, P), dtype=f32)).T.copy(),
        "negm": _make_negm(),
    }
    qb = be1 @ Wq2
    kb = be1 @ Wk2
    vb = be1 @ Wv2
    b1e = be2 @ W1 + b1
    out["qb"] = np.ascontiguousarray(qb.reshape(CCH, P).T).astype(f32)
    out["kb"] = np.ascontiguousarray(kb.reshape(CCH, P).T).astype(f32)
    out["vb"] = vb[None, :].astype(f32)
    out["bo"] = bo[None, :].astype(f32)
    out["b1c"] = np.ascontiguousarray(b1e.reshape(FCH, P).T).astype(f32)
    out["b2"] = b2[None, :].astype(f32)
    flags = set()
    for name, vec in (("qb", qb), ("kb", kb), ("vb", vb),
                      ("bo", bo), ("b1", b1e), ("b2", b2)):
        if np.any(vec != 0):
            flags.add(name)
    return out, frozenset(flags)


_PROGRAM_CACHE = {}


def _get_program(bl, flags):
    key = (bl, flags)
    if key not in _PROGRAM_CACHE:
        _PROGRAM_CACHE[key] = build_program(
            bl, flags, tr_split=True, ps1_bufs=8)
    return _PROGRAM_CACHE[key]


def kernel(x, Wq, Wk, Wv, Wo, bo, W1, b1, W2, b2, g1, be1, g2, be2, **kw):
    from concourse.bass_utils import run_bass_kernel_spmd

    args = [np.asarray(a, dtype=np.float32) for a in
            (x, Wq, Wk, Wv, Wo, bo, W1, b1, W2, b2, g1, be1, g2, be2)]
    x = args[0]
    wmap, flags = prep_weights(*args[1:])
    nc = _get_program(BL, flags)
    xs = x.reshape(NCORES, BL, T, C)
    in_maps = []
    for c in range(NCORES):
        m = {"x": np.ascontiguousarray(xs[c])}
        m.update(wmap)
        in_maps.append(m)
    res = run_bass_kernel_spmd(nc, in_maps, list(range(NCORES)), **kw)
    global _last_results
    _last_results = res
    y = np.stack([res.results[i]["y"] for i in range(NCORES)], axis=0)
    return y.reshape(B, T, C)


_last_results = None

